# revision 1
# baseline (speedup 1.0000x reference)
"""EnhancedCrossAttention TRN2 kernel.

Strategy: data-parallel over batch B=2048 across 8 cores (256 rows each),
weights replicated, no collectives.

Per core (b_loc=256, two b-tiles of 128):
  q/k/v projections as fp16 matmuls (fp32 PSUM accumulate) with the
  activation tiles (contraction-major, pre-transposed on host) stationary
  and the weight matrix streaming as the moving operand, output layout
  [b partitions, feature free].  pos_encoding is folded on the host into a
  per-(t, o) bias (pos @ Wk.T + bk) added on PSUM eviction.  K and V never
  leave the chip: raw scores for all 16 heads are computed inline per
  k row-tile (one q*k multiply + per-head reduce on DVE), softmax uses a
  fused exp+row-sum on the scalar engine, and AV accumulates directly from
  the V PSUM via fused (pv * p) + acc scalar_tensor_tensor ops.  Attention
  output is PE-transposed and fed to the o-projection (weights streamed in
  chunks).  Wq/Wo stream as 512-wide chunks; only Wk/Wv occupy the big
  weight slot.
"""

import numpy as np

import concourse.bass as bass
import concourse.mybir as mybir
import concourse.tile as tile
from concourse import bacc
from concourse.bass_utils import run_bass_kernel_spmd
from concourse.masks import make_identity

B, T, D = 2048, 32, 2048
H, HD = 16, 128
NCORES = 8
BL = B // NCORES  # 256 batch rows per core

FP16 = mybir.dt.float16
FP32 = mybir.dt.float32

ITILES = D // 128   # 16 contraction tiles
OCH = D // 512      # 4 output chunks of 512 (one PSUM bank each)
INV_SQRT_HD = 1.0 / float(np.sqrt(HD))


def build_nc(b_loc=BL, nreps=1):
    nbt = b_loc // 128
    nc = bacc.Bacc("TRN2", target_bir_lowering=False, debug=False)

    queryT = nc.dram_tensor("queryT", [D, b_loc], FP16, kind="ExternalInput")
    keysT = nc.dram_tensor("keysT", [T, D, b_loc], FP16, kind="ExternalInput")
    valuesT = nc.dram_tensor("valuesT", [T, D, b_loc], FP16, kind="ExternalInput")
    wqT = nc.dram_tensor("wqT", [D, D], FP16, kind="ExternalInput")
    wkT = nc.dram_tensor("wkT", [D, D], FP16, kind="ExternalInput")
    wvT = nc.dram_tensor("wvT", [D, D], FP16, kind="ExternalInput")
    woT = nc.dram_tensor("woT", [D, D], FP16, kind="ExternalInput")
    pos_bias = nc.dram_tensor("pos_bias", [T, D], FP16, kind="ExternalInput")
    out = nc.dram_tensor("out", [b_loc, D], FP32, kind="ExternalOutput")

    X = mybir.AxisListType.X
    MULT = mybir.AluOpType.mult
    ADD = mybir.AluOpType.add

    with tile.TileContext(nc) as tc:
        with (
            tc.tile_pool(name="consts", bufs=1) as consts,
            tc.tile_pool(name="wpool", bufs=1) as wpool,
            tc.tile_pool(name="wqs", bufs=8) as wqs_pool,
            tc.tile_pool(name="iopool", bufs=1) as iopool,
            tc.tile_pool(name="lhst", bufs=3) as lhst_pool,
            tc.tile_pool(name="evict", bufs=3) as evict_pool,
            tc.tile_pool(name="posb", bufs=2) as pos_pool,
            tc.tile_pool(name="prod", bufs=2) as prod_pool,
            tc.tile_pool(name="small", bufs=4) as small_pool,
            tc.tile_pool(name="aot", bufs=2) as aot_pool,
        ):
            ident = consts.tile([128, 128], FP16)
            make_identity(nc, ident)

            for rep in range(nreps):
                qT_sb = iopool.tile([128, ITILES, b_loc], FP16, tag="qT",
                                    name="qT_sb")
                nc.sync.dma_start(
                    out=qT_sb,
                    in_=queryT.ap().rearrange("(a p) b -> p a b", p=128),
                )
                q_sb = iopool.tile([128, nbt, D], FP16, tag="q", name="q_sb")
                # raw scores [b, t, h], filled one t-slice per k row-tile
                sc = [
                    iopool.tile([128, T, H], FP32, tag=f"sc{bt}",
                                name=f"sc{bt}")
                    for bt in range(nbt)
                ]
                p_all = [
                    iopool.tile([128, H, T], FP32, tag=f"p{bt}",
                                name=f"p_all{bt}")
                    for bt in range(nbt)
                ]
                rs_all = [
                    iopool.tile([128, H], FP32, tag=f"rs{bt}",
                                name=f"rs_all{bt}")
                    for bt in range(nbt)
                ]
                # fp32 AV accumulators (one per b-tile), normalized+cast at
                # the end into attnout16
                acc = [
                    iopool.tile([128, D], FP32, tag=f"acc{bt}",
                                name=f"acc{bt}")
                    for bt in range(nbt)
                ]
                attnout = [
                    iopool.tile([128, D], FP16, tag=f"ao{bt}",
                                name=f"attnout{bt}")
                    for bt in range(nbt)
                ]

                def load_weight(w_dram):
                    w_sb = wpool.tile([128, ITILES, D], FP16, tag="w",
                                      name="w_sb")
                    nc.sync.dma_start(
                        out=w_sb,
                        in_=w_dram.ap().rearrange("(a p) o -> p a o", p=128),
                    )
                    return w_sb

                def load_wchunk(w_dram, it, occ):
                    wc = wqs_pool.tile([128, 512], FP16, tag="wqc",
                                       name="w_c")
                    nc.sync.dma_start(
                        out=wc,
                        in_=w_dram.ap()[
                            it * 128:(it + 1) * 128, occ * 512:(occ + 1) * 512
                        ],
                    )
                    return wc

                def load_lhsT(src_2d, bt):
                    lt = lhst_pool.tile([128, ITILES, 128], FP16, tag="lhsT",
                                        name="lt")
                    nc.sync.dma_start(
                        out=lt,
                        in_=src_2d.rearrange("(a p) b -> p a b", p=128)[
                            :, :, bt * 128:(bt + 1) * 128
                        ],
                    )
                    return lt

                with tc.tile_pool(name="psA", bufs=2, space="PSUM") as psA:
                    # ---- q-projection (weights streamed in chunks so the
                    # big slot is free for wk prefetch) ----
                    pq = [
                        psA.tile([128, D], FP32, tag="pk", name=f"pq{bt}")
                        for bt in range(nbt)
                    ]
                    for it in range(ITILES):
                        for oc in range(OCH):
                            wq_c = load_wchunk(wqT, it, oc)
                            for bt in range(nbt):
                                nc.tensor.matmul(
                                    pq[bt][:, oc * 512:(oc + 1) * 512],
                                    qT_sb[:, it, bt * 128:(bt + 1) * 128],
                                    wq_c,
                                    start=(it == 0),
                                    stop=(it == ITILES - 1),
                                )
                    for bt in range(nbt):
                        nc.scalar.copy(q_sb[:, bt, :], pq[bt])

                    # ---- k-projection with inline scores: k never leaves
                    # SBUF.  Per row-tile: k = psum + pos bias, then one
                    # q*k multiply and a per-head reduce give the raw
                    # scores for all 16 heads at this t. ----
                    wk_sb = load_weight(wkT)
                    for bt in range(nbt):
                        for t in range(T):
                            lt = load_lhsT(keysT.ap()[t], bt)
                            pos_bc = pos_pool.tile([128, D], FP16, tag="pos",
                                                   name="pos_bc")
                            nc.sync.dma_start(
                                out=pos_bc,
                                in_=pos_bias.ap()[t:t + 1, :].broadcast_to(
                                    (128, D)
                                ),
                            )
                            pk = psA.tile([128, D], FP32, tag="pk", name="pk")
                            for it in range(ITILES):
                                for oc in range(OCH):
                                    nc.tensor.matmul(
                                        pk[:, oc * 512:(oc + 1) * 512],
                                        lt[:, it, :],
                                        wk_sb[:, it, oc * 512:(oc + 1) * 512],
                                        start=(it == 0),
                                        stop=(it == ITILES - 1),
                                    )
                            k_sb = evict_pool.tile([128, D], FP16, tag="ev",
                                                   name="k_sb")
                            nc.vector.tensor_tensor(
                                out=k_sb, in0=pk, in1=pos_bc, op=ADD,
                            )
                            sprod = prod_pool.tile([128, D], FP16,
                                                   tag="prod", name="sprod")
                            nc.vector.tensor_tensor(
                                out=sprod, in0=q_sb[:, bt, :], in1=k_sb,
                                op=MULT,
                            )
                            nc.vector.tensor_reduce(
                                out=sc[bt][:, t, :],
                                in_=sprod.rearrange("p (h d) -> p h d", h=H),
                                axis=X,
                                op=ADD,
                            )
                        # softmax for this b-tile (overlaps next phases)
                        for h in range(H):
                            smax = small_pool.tile([128, 1], FP32, tag="smax",
                                                   name="smax")
                            nc.vector.tensor_reduce(
                                out=smax, in_=sc[bt][:, :, h], axis=X,
                                op=mybir.AluOpType.max,
                            )
                            negmax = small_pool.tile([128, 1], FP32,
                                                     tag="negmax",
                                                     name="negmax")
                            nc.vector.tensor_scalar_mul(
                                negmax, smax, -INV_SQRT_HD
                            )
                            se = small_pool.tile([128, 1], FP32, tag="se",
                                                 name="se")
                            nc.scalar.activation(
                                p_all[bt][:, h, :],
                                sc[bt][:, :, h],
                                mybir.ActivationFunctionType.Exp,
                                bias=negmax,
                                scale=INV_SQRT_HD,
                                accum_out=se,
                            )
                            nc.vector.reciprocal(rs_all[bt][:, h:h + 1], se)

                # psA closed; v-proj + o-proj use separate PSUM pools
                wv_sb = load_weight(wvT)
                with (
                    tc.tile_pool(name="psV", bufs=2, space="PSUM") as psV,
                    tc.tile_pool(name="psB", bufs=1, space="PSUM") as psB,
                ):
                    for bt in range(nbt):
                        # v-projection in head-halves; AV accumulates
                        # directly from PSUM via fused (pv*p)+acc, so v
                        # never leaves the chip either
                        for oh in range(2):
                            for t in range(T):
                                lt = load_lhsT(valuesT.ap()[t], bt)
                                pv = psV.tile([128, D // 2], FP32, tag="pv",
                                              name="pv")
                                for it in range(ITILES):
                                    for oc in range(2):
                                        occ = oh * 2 + oc
                                        nc.tensor.matmul(
                                            pv[:, oc * 512:(oc + 1) * 512],
                                            lt[:, it, :],
                                            wv_sb[:, it,
                                                  occ * 512:(occ + 1) * 512],
                                            start=(it == 0),
                                            stop=(it == ITILES - 1),
                                        )
                                for hh in range(8):
                                    h = oh * 8 + hh
                                    hsl = slice(h * HD, (h + 1) * HD)
                                    psl = pv[:, hh * HD:(hh + 1) * HD]
                                    pcol = p_all[bt][:, h, t:t + 1]
                                    if t == 0:
                                        nc.vector.tensor_scalar_mul(
                                            acc[bt][:, hsl], psl, pcol
                                        )
                                    else:
                                        nc.vector.scalar_tensor_tensor(
                                            out=acc[bt][:, hsl],
                                            in0=psl,
                                            scalar=pcol,
                                            in1=acc[bt][:, hsl],
                                            op0=MULT,
                                            op1=ADD,
                                        )
                        # normalize by 1/sum(exp) and cast to fp16
                        for h in range(H):
                            hsl = slice(h * HD, (h + 1) * HD)
                            nc.vector.tensor_scalar_mul(
                                attnout[bt][:, hsl], acc[bt][:, hsl],
                                rs_all[bt][:, h:h + 1],
                            )
                        # o-projection for this b-tile (wo streamed in
                        # chunks: no second big weight slot needed)
                        aoT = aot_pool.tile([128, ITILES, 128], FP16,
                                            tag="aoT", name="aoT")
                        for it in range(ITILES):
                            pt = psB.tile([128, 128], FP16, tag="pt",
                                          bufs=2, name="pt")
                            nc.tensor.transpose(
                                pt, attnout[bt][:, it * 128:(it + 1) * 128],
                                ident,
                            )
                            nc.scalar.copy(aoT[:, it, :], pt)
                        for half in range(2):
                            po = psB.tile([128, D // 2], FP32, tag="po",
                                          bufs=1, name="po")
                            for it in range(ITILES):
                                for oc in range(2):
                                    occ = half * 2 + oc
                                    wo_c = load_wchunk(woT, it, occ)
                                    nc.tensor.matmul(
                                        po[:, oc * 512:(oc + 1) * 512],
                                        aoT[:, it, :],
                                        wo_c,
                                        start=(it == 0),
                                        stop=(it == ITILES - 1),
                                    )
                            out_sb = evict_pool.tile(
                                [128, D // 2], FP32, tag="osb", bufs=2,
                                name="out_sb"
                            )
                            nc.scalar.copy(out_sb, po)
                            nc.sync.dma_start(
                                out=out.ap()[
                                    bt * 128:(bt + 1) * 128,
                                    half * 1024:(half + 1) * 1024,
                                ],
                                in_=out_sb,
                            )

    nc.compile()
    return nc


def host_prep(query, keys, values, mask, pos_encoding, Wq, bq, Wk, bk, Wv, bv,
              Wo, bo):
    """Build per-core input maps.  All heavy tensors pre-transposed to
    contraction-major layout and cast to fp16 on the host."""
    query = np.asarray(query, dtype=np.float32)
    keys = np.asarray(keys, dtype=np.float32)
    values = np.asarray(values, dtype=np.float32)
    pos_encoding = np.asarray(pos_encoding, dtype=np.float32)
    Wq, Wk, Wv, Wo = (np.asarray(w, dtype=np.float32) for w in (Wq, Wk, Wv, Wo))
    bk = np.asarray(bk, dtype=np.float32)

    wqT = np.ascontiguousarray(Wq.T).astype(np.float16)
    wkT = np.ascontiguousarray(Wk.T).astype(np.float16)
    wvT = np.ascontiguousarray(Wv.T).astype(np.float16)
    woT = np.ascontiguousarray(Wo.T).astype(np.float16)

    pos = np.clip(pos_encoding[:T], -10.0, 10.0)
    pos_bias = (pos @ Wk.T + bk).astype(np.float16)  # (T, D)

    in_maps = []
    for c in range(NCORES):
        sl = slice(c * BL, (c + 1) * BL)
        in_maps.append({
            "queryT": np.ascontiguousarray(query[sl].T).astype(np.float16),
            "keysT": np.ascontiguousarray(
                keys[:, sl, :].transpose(0, 2, 1)).astype(np.float16),
            "valuesT": np.ascontiguousarray(
                values[:, sl, :].transpose(0, 2, 1)).astype(np.float16),
            "wqT": wqT, "wkT": wkT, "wvT": wvT, "woT": woT,
            "pos_bias": pos_bias,
        })
    return in_maps


_STATE = {}


def _get_nc():
    if "nc" not in _STATE:
        _STATE["nc"] = build_nc()
    return _STATE["nc"]


def run_on_hw(in_maps, trace=False):
    nc = _get_nc()
    return run_bass_kernel_spmd(nc, in_maps, list(range(NCORES)), trace=trace)


def kernel(**inputs):
    in_maps = host_prep(**inputs)
    res = run_on_hw(in_maps)
    return np.concatenate(
        [np.asarray(res.results[c]["out"]) for c in range(NCORES)], axis=0
    )



# revision 3
# speedup vs baseline: 1.0803x; 1.0803x over previous
"""EnhancedCrossAttention TRN2 kernel, v2.

Data-parallel over batch B=2048 across 8 cores (256 rows each), weights
replicated, no collectives.  All fp16 matmuls (fp8 fails the 2e-2 gate).

v2 changes vs v1 (all aimed at PE stalls + DMA efficiency; PE work is
already the roofline):
  - every DMA is contiguous per partition (host pre-lays tiles), killing
    descriptor-bound transfers (startup qT gather was ~2048 descriptors)
  - all four weight matrices live in one rotating 2-generation pool of 16
    per-it tiles: wq streams during startup, wk prefetches under Q-phase,
    wv under K-phase, wo under V-phase -> no phase-boundary DMA stalls,
    no per-chunk streaming starvation in Q/O projections
  - values are loaded once (single full-D PSUM pass, no oh split)
  - pos_encoding enters scores as per-(b,h,t) scalars computed by tiny PE
    matmuls (q_proj^T @ pos_biasT), replacing 33.5MB of broadcast DMAs and
    a DVE add per (bt,t); the scores multiply reads K directly from PSUM
"""

import numpy as np

import concourse.bass as bass
import concourse.mybir as mybir
import concourse.tile as tile
from concourse import bacc
from concourse.bass_utils import run_bass_kernel_spmd
from concourse.masks import make_identity

B, T, D = 2048, 32, 2048
H, HD = 16, 128
NCORES = 8
BL = B // NCORES  # 256 batch rows per core

FP16 = mybir.dt.float16
FP32 = mybir.dt.float32

ITILES = D // 128   # 16 contraction tiles
OCH = D // 512      # 4 output chunks of 512 (one PSUM bank each)
INV_SQRT_HD = 1.0 / float(np.sqrt(HD))


def build_nc(b_loc=BL, nreps=1):
    nbt = b_loc // 128
    nc = bacc.Bacc("TRN2", target_bir_lowering=False, debug=False)

    # host-prepped contiguous layouts (see host_prep)
    queryT = nc.dram_tensor("queryT", [128, ITILES, b_loc], FP16,
                            kind="ExternalInput")
    keysT = nc.dram_tensor("keysT", [T, nbt, 128, ITILES, 128], FP16,
                           kind="ExternalInput")
    valuesT = nc.dram_tensor("valuesT", [T, nbt, 128, ITILES, 128], FP16,
                             kind="ExternalInput")
    wqT = nc.dram_tensor("wqT", [128, ITILES, D], FP16, kind="ExternalInput")
    wkT = nc.dram_tensor("wkT", [128, ITILES, D], FP16, kind="ExternalInput")
    wvT = nc.dram_tensor("wvT", [128, ITILES, D], FP16, kind="ExternalInput")
    woT = nc.dram_tensor("woT", [128, ITILES, D], FP16, kind="ExternalInput")
    posT = nc.dram_tensor("posT", [128, H, T], FP16, kind="ExternalInput")
    out = nc.dram_tensor("out", [b_loc, D], FP32, kind="ExternalOutput")

    X = mybir.AxisListType.X
    MULT = mybir.AluOpType.mult
    ADD = mybir.AluOpType.add

    with tile.TileContext(nc) as tc:
        with (
            tc.tile_pool(name="consts", bufs=1) as consts,
            tc.tile_pool(name="wpool", bufs=2) as wpool,
            tc.tile_pool(name="iopool", bufs=1) as iopool,
            tc.tile_pool(name="lhst", bufs=2) as lhst_pool,
            tc.tile_pool(name="evict", bufs=2) as evict_pool,
            tc.tile_pool(name="prod", bufs=2) as prod_pool,
            tc.tile_pool(name="small", bufs=4) as small_pool,
        ):
            ident = consts.tile([128, 128], FP16)
            make_identity(nc, ident)

            def load_w16(w_dram):
                """16 per-it tiles; generation rotation gives prefetch-
                behind-current-phase for free.  Even/odd its go to the two
                HWDGE queues (SP / Activation) to double burst bandwidth
                and keep lhsT loads unblocked on the SP queue."""
                tiles = []
                for it in range(ITILES):
                    wt = wpool.tile([128, D], FP16, tag=f"w{it}",
                                    name=f"w{it}")
                    nc.sync.dma_start(out=wt, in_=w_dram.ap()[:, it, :])
                    tiles.append(wt)
                return tiles

            for rep in range(nreps):
                qT_sb = iopool.tile([128, ITILES, b_loc], FP16, tag="qT",
                                    name="qT_sb")
                nc.sync.dma_start(out=qT_sb, in_=queryT.ap())
                posT_sb = iopool.tile([128, H, T], FP16, tag="posT",
                                      name="posT_sb")
                nc.scalar.dma_start(out=posT_sb, in_=posT.ap())
                wq = load_w16(wqT)

                q_sb = iopool.tile([128, nbt, D], FP16, tag="q", name="q_sb")
                qT_proj = iopool.tile([128, nbt, H, HD], FP16, tag="qTp",
                                      name="qT_proj")
                sc = [
                    iopool.tile([128, T, H], FP32, tag=f"sc{bt}",
                                name=f"sc{bt}")
                    for bt in range(nbt)
                ]
                sc_pos = iopool.tile([128, nbt, H, T], FP32, tag="scp",
                                     name="sc_pos")
                p_all = [
                    iopool.tile([128, H, T], FP32, tag=f"p{bt}",
                                name=f"p_all{bt}")
                    for bt in range(nbt)
                ]
                rs_all = [
                    iopool.tile([128, H], FP32, tag=f"rs{bt}",
                                name=f"rs_all{bt}")
                    for bt in range(nbt)
                ]
                acc = [
                    iopool.tile([128, D], FP32, tag=f"acc{bt}",
                                name=f"acc{bt}")
                    for bt in range(nbt)
                ]

                # ---- Q phase: q-projection, it-major so each wq tile is
                # consumed as it lands ----
                with tc.tile_pool(name="psQ", bufs=1, space="PSUM") as psQ:
                    pq = [
                        psQ.tile([128, D], FP32, tag=f"pq{bt}",
                                 name=f"pq{bt}")
                        for bt in range(nbt)
                    ]
                    wk = load_w16(wkT)  # prefetch under Q compute
                    for it in range(ITILES):
                        for bt in range(nbt):
                            for oc in range(OCH):
                                nc.tensor.matmul(
                                    pq[bt][:, oc * 512:(oc + 1) * 512],
                                    qT_sb[:, it, bt * 128:(bt + 1) * 128],
                                    wq[it][:, oc * 512:(oc + 1) * 512],
                                    start=(it == 0),
                                    stop=(it == ITILES - 1),
                                )
                    for bt in range(nbt):
                        nc.scalar.copy(q_sb[:, bt, :], pq[bt])

                # ---- q transposes + pos-scores:
                # sc_pos[b, h, t] = q_proj[b, hslice] . pos_bias[t, hslice]
                with (
                    tc.tile_pool(name="psT", bufs=2, space="PSUM") as psT,
                    tc.tile_pool(name="psP", bufs=1, space="PSUM") as psP,
                ):
                    for bt in range(nbt):
                        for h in range(H):
                            pt = psT.tile([128, 128], FP16, tag="pt",
                                          name="pt")
                            nc.tensor.transpose(
                                pt, q_sb[:, bt, h * HD:(h + 1) * HD], ident
                            )
                            nc.scalar.copy(qT_proj[:, bt, h, :], pt)
                    for bt in range(nbt):
                        pps = psP.tile([128, H * T], FP32, tag=f"pps{bt}",
                                       name=f"pps{bt}")
                        for h in range(H):
                            nc.tensor.matmul(
                                pps[:, h * T:(h + 1) * T],
                                qT_proj[:, bt, h, :],
                                posT_sb[:, h, :],
                                start=True, stop=True,
                            )
                        nc.scalar.copy(
                            sc_pos.rearrange("p b h t -> p (b h t)")[
                                :, bt * H * T:(bt + 1) * H * T
                            ],
                            pps,
                        )

                # ---- K phase: k never leaves PSUM; scores for all 16
                # heads per (bt, t) via q*k multiplies + per-head reduces.
                # Half-D pk tiles (bufs=3) so the score DVE work for half 0
                # overlaps the half-1 matmuls and the tile-boundary chain
                # is a 1.1us sprod, not 2.3us.
                with tc.tile_pool(name="psA", bufs=3, space="PSUM") as psA:
                    wv = None
                    for bt in range(nbt):
                        for t in range(T):
                            lt = lhst_pool.tile([128, ITILES, 128], FP16,
                                                tag="lhsT", name="lt")
                            nc.sync.dma_start(out=lt, in_=keysT.ap()[t, bt])
                            if wv is None and t == 2:
                                wv = load_w16(wvT)  # prefetch under K
                            for oh in range(2):
                                pk = psA.tile([128, D // 2], FP32, tag="pk",
                                              name="pk")
                                for it in range(ITILES):
                                    for oc in range(2):
                                        occ = oh * 2 + oc
                                        nc.tensor.matmul(
                                            pk[:, oc * 512:(oc + 1) * 512],
                                            lt[:, it, :],
                                            wk[it][:, occ * 512:(occ + 1) * 512],
                                            start=(it == 0),
                                            stop=(it == ITILES - 1),
                                        )
                                sprod = prod_pool.tile([128, D // 2], FP16,
                                                       tag="prod",
                                                       name="sprod")
                                nc.vector.tensor_tensor(
                                    out=sprod,
                                    in0=q_sb[:, bt,
                                             oh * (D // 2):(oh + 1) * (D // 2)],
                                    in1=pk,
                                    op=MULT,
                                )
                                nc.vector.tensor_reduce(
                                    out=sc[bt][:, t,
                                               oh * (H // 2):(oh + 1) * (H // 2)],
                                    in_=sprod.rearrange(
                                        "p (h d) -> p h d", h=H // 2
                                    ),
                                    axis=X,
                                    op=ADD,
                                )
                        # fold pos scores in, then softmax for this b-tile
                        for h in range(H):
                            nc.vector.tensor_tensor(
                                out=sc[bt][:, :, h],
                                in0=sc[bt][:, :, h],
                                in1=sc_pos[:, bt, h, :],
                                op=ADD,
                            )
                            smax = small_pool.tile([128, 1], FP32,
                                                   tag="smax", name="smax")
                            nc.vector.tensor_reduce(
                                out=smax, in_=sc[bt][:, :, h], axis=X,
                                op=mybir.AluOpType.max,
                            )
                            negmax = small_pool.tile([128, 1], FP32,
                                                     tag="negmax",
                                                     name="negmax")
                            nc.vector.tensor_scalar_mul(
                                negmax, smax, -INV_SQRT_HD
                            )
                            se = small_pool.tile([128, 1], FP32, tag="se",
                                                 name="se")
                            nc.scalar.activation(
                                p_all[bt][:, h, :],
                                sc[bt][:, :, h],
                                mybir.ActivationFunctionType.Exp,
                                bias=negmax,
                                scale=INV_SQRT_HD,
                                accum_out=se,
                            )
                            nc.vector.reciprocal(rs_all[bt][:, h:h + 1], se)

                # ---- V phase: single full-D pass; AV accumulates straight
                # from PSUM via fused (pv * p) + acc.  Each bt's attnout is
                # normalized and DMA-transposed (XBAR) while the next bt's
                # V pass runs, so the O phase starts on matmuls directly ----
                attnout = iopool.tile([128, nbt, D], FP16, tag="q",
                                      name="attnout")
                # per-bt tiles so O(bt0) doesn't wait on bt1's transposes
                aoT = [
                    iopool.tile([128, ITILES, 128], FP16, tag=f"aoT{bt}",
                                name=f"aoT{bt}")
                    for bt in range(nbt)
                ]
                # pv is half-D so psV (3x2 banks) and psB (2x1) coexist:
                # no pool barrier between V and O, the o-projection chains
                # PE-continuously behind the last V matmul
                def emit_oproj(bt):
                    for oc in range(OCH):
                        po = psB.tile([128, 512], FP32, tag="po", name="po")
                        for it in range(ITILES):
                            nc.tensor.matmul(
                                po,
                                aoT[bt][:, it, :],
                                wo[it][:, oc * 512:(oc + 1) * 512],
                                start=(it == 0),
                                stop=(it == ITILES - 1),
                            )
                        out_sb = evict_pool.tile(
                            [128, 512], FP32, tag="osb", name="out_sb"
                        )
                        nc.scalar.copy(out_sb, po)
                        nc.scalar.dma_start(
                            out=out.ap()[
                                bt * 128:(bt + 1) * 128,
                                oc * 512:(oc + 1) * 512,
                            ],
                            in_=out_sb,
                        )

                with (
                    tc.tile_pool(name="psV", bufs=3, space="PSUM") as psV,
                    tc.tile_pool(name="psB", bufs=2, space="PSUM") as psB,
                ):
                    wo = None
                    for bt in range(nbt):
                        for t in range(T):
                            lt = lhst_pool.tile([128, ITILES, 128], FP16,
                                                tag="lhsT", name="lt")
                            nc.sync.dma_start(out=lt, in_=valuesT.ap()[t, bt])
                            if wo is None and t == 2:
                                wo = load_w16(woT)  # prefetch under V
                            for oh in range(2):
                                pv = psV.tile([128, D // 2], FP32, tag="pv",
                                              name="pv")
                                for it in range(ITILES):
                                    for oc in range(2):
                                        occ = oh * 2 + oc
                                        nc.tensor.matmul(
                                            pv[:, oc * 512:(oc + 1) * 512],
                                            lt[:, it, :],
                                            wv[it][:, occ * 512:(occ + 1) * 512],
                                            start=(it == 0),
                                            stop=(it == ITILES - 1),
                                        )
                                for hh in range(H // 2):
                                    h = oh * (H // 2) + hh
                                    hsl = slice(h * HD, (h + 1) * HD)
                                    psl = pv[:, hh * HD:(hh + 1) * HD]
                                    pcol = p_all[bt][:, h, t:t + 1]
                                    if t == 0:
                                        nc.vector.tensor_scalar_mul(
                                            acc[bt][:, hsl], psl, pcol
                                        )
                                    else:
                                        nc.vector.scalar_tensor_tensor(
                                            out=acc[bt][:, hsl],
                                            in0=psl,
                                            scalar=pcol,
                                            in1=acc[bt][:, hsl],
                                            op0=MULT,
                                            op1=ADD,
                                        )
                        # normalize + XBAR-transpose this bt under the next
                        # bt's V compute
                        for h in range(H):
                            hsl = slice(h * HD, (h + 1) * HD)
                            nc.vector.tensor_scalar_mul(
                                attnout[:, bt, hsl], acc[bt][:, hsl],
                                rs_all[bt][:, h:h + 1],
                            )
                        for it in range(ITILES):
                            nc.sync.dma_start_transpose(
                                aoT[bt][:, it, :],
                                attnout[:, bt, it * 128:(it + 1) * 128],
                            )
                        if bt > 0:
                            emit_oproj(bt - 1)
                    emit_oproj(nbt - 1)

    nc.compile()
    return nc


def host_prep(query, keys, values, mask, pos_encoding, Wq, bq, Wk, bk, Wv, bv,
              Wo, bo):
    """Per-core input maps; every kernel DMA source is contiguous."""
    query = np.asarray(query, dtype=np.float32)
    keys = np.asarray(keys, dtype=np.float32)
    values = np.asarray(values, dtype=np.float32)
    pos_encoding = np.asarray(pos_encoding, dtype=np.float32)
    Wq, Wk, Wv, Wo = (np.asarray(w, dtype=np.float32) for w in (Wq, Wk, Wv, Wo))
    bk = np.asarray(bk, dtype=np.float32)

    def wprep(Wx):
        # [p, it, o] with p the within-it contraction index
        return np.ascontiguousarray(
            Wx.T.reshape(ITILES, 128, D).transpose(1, 0, 2)
        ).astype(np.float16)

    wqT, wkT, wvT, woT = (wprep(w) for w in (Wq, Wk, Wv, Wo))

    pos = np.clip(pos_encoding[:T], -10.0, 10.0)
    pos_bias = pos @ Wk.T + bk                       # (T, D)
    posT = np.ascontiguousarray(
        pos_bias.T.reshape(H, HD, T).transpose(1, 0, 2)
    ).astype(np.float16)                             # (HD, H, T)

    def actprep(x_sl):
        # (T, BL, D) -> [t, bt, p, it, b] contiguous
        return np.ascontiguousarray(
            x_sl.reshape(T, BL // 128, 128, ITILES, 128).transpose(0, 1, 4, 3, 2)
        ).astype(np.float16)

    in_maps = []
    for c in range(NCORES):
        sl = slice(c * BL, (c + 1) * BL)
        qT = np.ascontiguousarray(
            query[sl].T.reshape(ITILES, 128, BL).transpose(1, 0, 2)
        ).astype(np.float16)
        in_maps.append({
            "queryT": qT,
            "keysT": actprep(keys[:, sl, :]),
            "valuesT": actprep(values[:, sl, :]),
            "wqT": wqT, "wkT": wkT, "wvT": wvT, "woT": woT,
            "posT": posT,
        })
    return in_maps


_STATE = {}


def _get_nc():
    if "nc" not in _STATE:
        _STATE["nc"] = build_nc()
    return _STATE["nc"]


def run_on_hw(in_maps, trace=False):
    nc = _get_nc()
    return run_bass_kernel_spmd(nc, in_maps, list(range(NCORES)), trace=trace)


def kernel(**inputs):
    in_maps = host_prep(**inputs)
    res = run_on_hw(in_maps)
    return np.concatenate(
        [np.asarray(res.results[c]["out"]) for c in range(NCORES)], axis=0
    )
